# revision 18
# baseline (speedup 1.0000x reference)
"""Trainium2 Bass kernel for the periodic H8 FE-conv operator.

Computation (reference semantics):
    Ue[x,y,z,b]   = U[(x+db)%, (y+db)%, (z+db)%]           (8 corner gather)
    Ve[...,a]     = sum_b filters[H8types][a,b] * Ue[...,b]
    V[n]          = sum_a Ve[n - da, a]                     (scatter, periodic)

Algebraic form used here (T=2 types, Ke_m = f0 + m*df):
    V[n] = sum_c W0[c] U[n+c]              (fixed 27-tap stencil from f0, "A0")
         + sum_b (Ptilde_b (*) W_b)[n]     (mask term)
    W_b[w]    = m[w] * U[w + db]           (8 masked-gather fields)
    Ptilde_b  = sum_a df[a,b] S_{-da}      (8-tap scatter stencil per b)

Mapping to TRN2 (per core, x-slab of 16 planes, 8 cores):
    layout [y=128 partitions, (x-plane, z) free].  y-shifts are circulant
    128x128 float32r matmul weights (lhsT); x/z shifts are AP offsets into
    host-padded slabs; the y+1 gather shift inside W_b uses a host-provided
    pre-shifted copy of the u slab.  W_b multiplies are SBUF*SBUF elementwise
    ops split across DVE and GPSIMD.  All matmuls accumulate in fp32 PSUM.
"""

import numpy as np

N = 128
NCORES = 8
SLAB = N // NCORES  # 16

CORNERS = np.array(
    [[0, 0, 0], [1, 0, 0], [0, 1, 0], [1, 1, 0],
     [0, 0, 1], [1, 0, 1], [0, 1, 1], [1, 1, 1]], dtype=np.int32)

_CIDX = {(int(d[0]), int(d[1]), int(d[2])): i for i, d in enumerate(CORNERS)}

# A0 groups: (dx, dz) pairs; dy in the 3-tap circulant
A0_GROUPS = [(dx, dz) for dx in (-1, 0, 1) for dz in (-1, 0, 1)]
# scatter groups: (dax, daz) pairs; day handled in the 2-tap circulant
PG = [(0, 0), (0, 1), (1, 0), (1, 1)]
# W-multiply plane chunks (storage idx s = local e + 1, e in [-1, 15])
W_CHUNKS = [(0, 5), (5, 4), (9, 4), (13, 4)]

U_COLS = N + 3    # z pad [-1..129], col j = z + 1
M_COLS = N + 1    # z pad [-1..127], col j = z + 1
W_MATS = 41

W_ELEMS = W_MATS * N
U_ELEMS = (SLAB + 2) * U_COLS
M_ELEMS = (SLAB + 1) * M_COLS
UOFF = W_ELEMS
UDOFF = UOFF + U_ELEMS
MOFF = UDOFF + U_ELEMS
TOT = MOFF + M_ELEMS


def _roll_mat(s):
    """lhsT[y_in, y_out] = 1 iff y_in == (y_out + s) mod 128."""
    return np.roll(np.eye(N, dtype=np.float64), s, axis=0)


def check_proportional(filters):
    """If filters[0] ~= rho * (filters[1]-filters[0]), return rho, else None."""
    f0 = filters[0].astype(np.float64)
    df = filters[1].astype(np.float64) - f0
    denom = float((df * df).sum())
    if denom == 0.0:
        return None
    rho = float((f0 * df).sum()) / denom
    resid = np.abs(f0 - rho * df).max()
    scale = max(np.abs(f0).max(), 1e-30)
    return rho if resid <= 1e-4 * max(scale, np.abs(df).max()) else None


def build_weights_a0(filters):
    """[128, 9, 128] f32 A0 lhsT stack (general-path only)."""
    f0 = filters[0].astype(np.float64)
    W0 = np.zeros((3, 3, 3))
    for a in range(8):
        for b in range(8):
            c = CORNERS[b] - CORNERS[a]
            W0[c[0] + 1, c[1] + 1, c[2] + 1] += f0[a, b]
    mats = []
    for dx, dz in A0_GROUPS:
        M = np.zeros((N, N))
        for dy in (-1, 0, 1):
            w = W0[dx + 1, dy + 1, dz + 1]
            if w != 0.0:
                M += w * _roll_mat(dy)
        mats.append(M)
    return np.ascontiguousarray(
        np.stack(mats).astype(np.float32).transpose(1, 0, 2))


def build_weights_p(filters):
    """[128, 32, 128] f32 Ptilde lhsT stack (b-major).

    Ptilde: out[y] += sum_day df[a(dax,day,daz), b] * W_b[y - day]
    """
    f0 = filters[0].astype(np.float64)
    df = filters[1].astype(np.float64) - f0
    mats = []
    for b in range(8):
        for dax, daz in PG:
            M = (df[_CIDX[(dax, 0, daz)], b] * _roll_mat(0)
                 + df[_CIDX[(dax, 1, daz)], b] * _roll_mat(-1))
            mats.append(M)
    return np.ascontiguousarray(
        np.stack(mats).astype(np.float32).transpose(1, 0, 2))


def build_slabs(U, H8types, mask_bias=0.0):
    """Per-core slab pairs (u, m), f32.

    u:  [128(y), 18(x local -1..16), 131(z pad -1..129)]
    m:  [128(y), 17(e local -1..15), 129(z pad -1..127)], values + mask_bias
    """
    m_full = H8types.astype(np.float32) + np.float32(mask_bias)
    out = []
    for c in range(NCORES):
        x0 = c * SLAB
        xi = (np.arange(x0 - 1, x0 + SLAB + 1)) % N          # 18 planes
        u = U[xi]                                            # [18,128,128]
        u = np.concatenate(
            [u[:, :, [N - 1]], u, u[:, :, [0]], u[:, :, [1]]], axis=2)
        u = np.ascontiguousarray(u.transpose(1, 0, 2))       # [128,18,131]

        ei = (np.arange(x0 - 1, x0 + SLAB)) % N              # 17 planes
        m = m_full[ei]
        m = np.concatenate([m[:, :, [N - 1]], m], axis=2)    # [17,128,129]
        m = np.ascontiguousarray(m.transpose(1, 0, 2))       # [128,17,129]
        out.append((u, m))
    return out


def build_program(use_a0, reps=1):
    """Trace the Bass/Tile program (shared across all 8 cores).

    The y+1-shifted u copy (ud) is built on device with 5 circulant shift
    matmuls + ScalarE PSUM->SBUF copies while the other DMAs stream in.
    When use_a0 is False the f0 stencil is folded into the mask bias
    (filters proportional case) and the A0 pass is skipped entirely.
    """
    import concourse.bacc as bacc
    import concourse.bass as bass
    import concourse.mybir as mybir
    import concourse.tile as tile

    f32 = mybir.dt.float32
    f32r = mybir.dt.float32r
    nc = bacc.Bacc("TRN2", target_bir_lowering=False, debug=False)

    wr_ext = nc.declare_dram_parameter("wr", [N, N], f32r, isOutput=False)
    u_ext = nc.declare_dram_parameter("u", [N, SLAB + 2, U_COLS], f32r, isOutput=False)
    m_ext = nc.declare_dram_parameter("m", [N, SLAB + 1, M_COLS], f32r, isOutput=False)
    wp_ext = nc.declare_dram_parameter("wp", [N, 32 * N], f32r, isOutput=False)
    if use_a0:
        wa_ext = nc.declare_dram_parameter("wa", [N, 9 * N], f32r, isOutput=False)
    v_ext = nc.declare_dram_parameter("v", [N, SLAB, N], f32, isOutput=True)

    # scatter iteration order: dby=0 fields (only need u) first
    B_ORDER = [0, 1, 4, 5, 2, 3, 6, 7]

    with tile.TileContext(nc) as tc:
        with (
            tc.tile_pool(name="const", bufs=1) as const,
            tc.tile_pool(name="wpool", bufs=1) as wpool,
            tc.tile_pool(name="psum", bufs=3, space=bass.MemorySpace.PSUM) as psum,
        ):
            wr_sb = const.tile([N, N], f32r, tag="wr")
            u_sb = const.tile([N, SLAB + 2, U_COLS], f32r, tag="u")
            m_sb = const.tile([N, SLAB + 1, M_COLS], f32r, tag="m")
            wp_sb = const.tile([N, 32 * N], f32r, tag="wp")
            ud_sb = const.tile([N, SLAB + 2, U_COLS], f32r, tag="ud")
            v_sb = const.tile([N, SLAB, N], f32, tag="v")

            nc.sync.dma_start(wr_sb[:], wr_ext[:])
            nc.scalar.dma_start(wp_sb[:], wp_ext[:])
            nc.sync.dma_start(u_sb[:], u_ext[:])
            if use_a0:
                wa_sb = const.tile([N, 9 * N], f32r, tag="wa")
                nc.scalar.dma_start(wa_sb[:], wa_ext[:])
            nc.sync.dma_start(m_sb[:], m_ext[:])

            W = [wpool.tile([N, SLAB + 1, M_COLS], f32r, tag=f"W{b}",
                            name=f"W{b}") for b in range(8)]
            uflat = u_sb.rearrange("p x z -> p (x z)")
            udflat = ud_sb.rearrange("p x z -> p (x z)")

            for rep in range(reps):
                # ---- ud[y] = u[y+1]: circulant shift matmuls ----
                c0 = 0
                while c0 < U_ELEMS:
                    cw = min(512, U_ELEMS - c0)
                    sps = psum.tile([N, cw], f32, tag="spsum",
                                    name=f"sps{rep}_{c0}")
                    nc.tensor.matmul(sps[:], wr_sb[:], uflat[:, c0:c0 + cw],
                                     start=True, stop=True)
                    nc.scalar.copy(udflat[:, c0:c0 + cw], sps[:])
                    c0 += cw

                # ---- W_b = m * U[.+db] (chunked; DVE / GPSIMD split) ----
                for ci, (s0, cnt) in enumerate(W_CHUNKS):
                    for b in B_ORDER:
                        dbx, dby, dbz = (int(v) for v in CORNERS[b])
                        usel = ud_sb if dby else u_sb
                        eng = nc.vector if b in (0, 1, 4, 2, 6) else nc.gpsimd
                        eng.tensor_mul(
                            W[b][:, s0:s0 + cnt, :],
                            m_sb[:, s0:s0 + cnt, :].bitcast(f32),
                            usel[:, s0 + dbx:s0 + dbx + cnt,
                                 dbz:dbz + M_COLS].bitcast(f32))

                # ---- V accumulation: [A0 stencil +] Ptilde scatter convs ----
                for oc in range(4):
                    vps = psum.tile([N, 4, N], f32, tag="vpsum",
                                    name=f"vps{rep}_{oc}")
                    first = True
                    if use_a0:
                        for gi, (dx, dz) in enumerate(A0_GROUPS):
                            rhs = u_sb[:, 4 * oc + 1 + dx:4 * oc + 5 + dx,
                                       dz + 1:dz + 1 + N]
                            nc.tensor.matmul(
                                vps[:], wa_sb[:, gi * N:(gi + 1) * N],
                                rhs, start=first, stop=False)
                            first = False
                    for bi, b in enumerate(B_ORDER):
                        for gidx, (dax, daz) in enumerate(PG):
                            i = b * 4 + gidx
                            rhs = W[b][:, 4 * oc - dax + 1:4 * oc - dax + 5,
                                       1 - daz:1 - daz + N]
                            nc.tensor.matmul(
                                vps[:], wp_sb[:, i * N:(i + 1) * N], rhs,
                                start=first,
                                stop=(bi == 7 and gidx == len(PG) - 1))
                            first = False
                    nc.scalar.copy(v_sb[:, 4 * oc:4 * oc + 4, :], vps[:])
                    if rep == reps - 1:
                        nc.sync.dma_start(v_ext[:, 4 * oc:4 * oc + 4, :],
                                          v_sb[:, 4 * oc:4 * oc + 4, :])

    nc.compile()
    return nc


_PROGRAM_CACHE = {}


def _get_program(use_a0):
    key = ("nc", use_a0)
    if key not in _PROGRAM_CACHE:
        _PROGRAM_CACHE[key] = build_program(use_a0)
    return _PROGRAM_CACHE[key]


_R_MAT = None


def kernel(U, H8types, filters, _trace=False):
    from concourse.bass_utils import run_bass_kernel_spmd

    U = np.asarray(U)
    H8types = np.asarray(H8types)
    filters = np.asarray(filters)

    global _R_MAT
    if _R_MAT is None:
        _R_MAT = np.ascontiguousarray(_roll_mat(1).astype(np.float32))

    rho = check_proportional(filters)
    use_a0 = rho is None
    slabs = build_slabs(U, H8types, mask_bias=0.0 if use_a0 else rho)
    wp = np.ascontiguousarray(build_weights_p(filters).reshape(N, -1))

    nc = _get_program(use_a0)
    core_ids = list(range(NCORES))
    in_maps = []
    for c in core_ids:
        u, m = slabs[c]
        im = {"wr": _R_MAT, "u": u, "m": m, "wp": wp}
        if use_a0:
            im["wa"] = np.ascontiguousarray(
                build_weights_a0(filters).reshape(N, -1))
        in_maps.append(im)

    res = run_bass_kernel_spmd(nc, in_maps, core_ids, trace=_trace)
    out = np.empty((N, N, N), dtype=np.float32)
    for c in core_ids:
        v = np.asarray(res.results[c]["v"])  # [128(y), 16(x), 128(z)]
        out[c * SLAB:(c + 1) * SLAB] = v.transpose(1, 0, 2)
    if _trace:
        return out, res
    return out


# revision 19
# speedup vs baseline: 2.6611x; 2.6611x over previous
"""Trainium2 Bass kernel for the periodic H8 FE-conv operator.

Computation (reference semantics):
    Ue[x,y,z,b]   = U[(x+db)%, (y+db)%, (z+db)%]           (8 corner gather)
    Ve[...,a]     = sum_b filters[H8types][a,b] * Ue[...,b]
    V[n]          = sum_a Ve[n - da, a]                     (scatter, periodic)

Algebraic form used here (T=2 types, Ke_m = f0 + m*df):
    V[n] = sum_c W0[c] U[n+c]              (fixed 27-tap stencil from f0, "A0")
         + sum_b (Ptilde_b (*) W_b)[n]     (mask term)
    W_b[w]    = m[w] * U[w + db]           (8 masked-gather fields)
    Ptilde_b  = sum_a df[a,b] S_{-da}      (8-tap scatter stencil per b)

Mapping to TRN2 (per core, x-slab of 16 planes, 8 cores):
    layout [y=128 partitions, (x-plane, z) free].  y-shifts are circulant
    128x128 float32r matmul weights (lhsT); x/z shifts are AP offsets into
    host-padded slabs; the y+1 gather shift inside W_b uses a host-provided
    pre-shifted copy of the u slab.  W_b multiplies are SBUF*SBUF elementwise
    ops split across DVE and GPSIMD.  All matmuls accumulate in fp32 PSUM.
"""

import numpy as np

N = 128
NCORES = 8
SLAB = N // NCORES  # 16

CORNERS = np.array(
    [[0, 0, 0], [1, 0, 0], [0, 1, 0], [1, 1, 0],
     [0, 0, 1], [1, 0, 1], [0, 1, 1], [1, 1, 1]], dtype=np.int32)

_CIDX = {(int(d[0]), int(d[1]), int(d[2])): i for i, d in enumerate(CORNERS)}

# A0 groups: (dx, dz) pairs; dy in the 3-tap circulant
A0_GROUPS = [(dx, dz) for dx in (-1, 0, 1) for dz in (-1, 0, 1)]
# scatter groups: (dax, daz) pairs; day handled in the 2-tap circulant
PG = [(0, 0), (0, 1), (1, 0), (1, 1)]
# W-multiply plane chunks (storage idx s = local e + 1, e in [-1, 15])
W_CHUNKS = [(0, 5), (5, 4), (9, 4), (13, 4)]

U_COLS = N + 3    # z pad [-1..129], col j = z + 1
M_COLS = N + 1    # z pad [-1..127], col j = z + 1
W_MATS = 41

W_ELEMS = W_MATS * N
U_ELEMS = (SLAB + 2) * U_COLS
M_ELEMS = (SLAB + 1) * M_COLS
UOFF = W_ELEMS
UDOFF = UOFF + U_ELEMS
MOFF = UDOFF + U_ELEMS
TOT = MOFF + M_ELEMS


def _roll_mat(s):
    """lhsT[y_in, y_out] = 1 iff y_in == (y_out + s) mod 128."""
    return np.roll(np.eye(N, dtype=np.float64), s, axis=0)


def check_proportional(filters):
    """If filters[0] ~= rho * (filters[1]-filters[0]), return rho, else None."""
    f0 = filters[0].astype(np.float64)
    df = filters[1].astype(np.float64) - f0
    denom = float((df * df).sum())
    if denom == 0.0:
        return None
    rho = float((f0 * df).sum()) / denom
    resid = np.abs(f0 - rho * df).max()
    scale = max(np.abs(f0).max(), 1e-30)
    return rho if resid <= 1e-4 * max(scale, np.abs(df).max()) else None


def build_weights_a0(filters):
    """[128, 9, 128] f32 A0 lhsT stack (general-path only)."""
    f0 = filters[0].astype(np.float64)
    W0 = np.zeros((3, 3, 3))
    for a in range(8):
        for b in range(8):
            c = CORNERS[b] - CORNERS[a]
            W0[c[0] + 1, c[1] + 1, c[2] + 1] += f0[a, b]
    mats = []
    for dx, dz in A0_GROUPS:
        M = np.zeros((N, N))
        for dy in (-1, 0, 1):
            w = W0[dx + 1, dy + 1, dz + 1]
            if w != 0.0:
                M += w * _roll_mat(dy)
        mats.append(M)
    return np.ascontiguousarray(
        np.stack(mats).astype(np.float32).transpose(1, 0, 2))


B_ORDER = [0, 1, 4, 5, 2, 3, 6, 7]  # dby=0 fields (only need u) first


def build_weights_p(filters):
    """[128, 32, 128] f32 Ptilde lhsT stack (B_ORDER-major).

    Ptilde: out[y] += sum_day df[a(dax,day,daz), b] * W_b[y - day]
    """
    f0 = filters[0].astype(np.float64)
    df = filters[1].astype(np.float64) - f0
    mats = []
    for b in B_ORDER:
        for dax, daz in PG:
            M = (df[_CIDX[(dax, 0, daz)], b] * _roll_mat(0)
                 + df[_CIDX[(dax, 1, daz)], b] * _roll_mat(-1))
            mats.append(M)
    return np.ascontiguousarray(
        np.stack(mats).astype(np.float32).transpose(1, 0, 2))


def build_slabs(U, H8types, mask_bias=0.0):
    """Per-core slab pairs (u, m), f32.

    u:  [128(y), 18(x local -1..16), 131(z pad -1..129)]
    m:  [128(y), 17(e local -1..15), 129(z pad -1..127)], values + mask_bias
    """
    m_full = H8types.astype(np.float32) + np.float32(mask_bias)
    out = []
    for c in range(NCORES):
        x0 = c * SLAB
        xi = (np.arange(x0 - 1, x0 + SLAB + 1)) % N          # 18 planes
        u = U[xi]                                            # [18,128,128]
        u = np.concatenate(
            [u[:, :, [N - 1]], u, u[:, :, [0]], u[:, :, [1]]], axis=2)
        u = np.ascontiguousarray(u.transpose(1, 0, 2))       # [128,18,131]

        ei = (np.arange(x0 - 1, x0 + SLAB)) % N              # 17 planes
        m = m_full[ei]
        m = np.concatenate([m[:, :, [N - 1]], m], axis=2)    # [17,128,129]
        m = np.ascontiguousarray(m.transpose(1, 0, 2))       # [128,17,129]
        out.append((u, m))
    return out


def build_program(use_a0, reps=1):
    """Trace the Bass/Tile program (shared across all 8 cores).

    The y+1-shifted u copy (ud) is built on device with 5 circulant shift
    matmuls + ScalarE PSUM->SBUF copies while the other DMAs stream in.
    When use_a0 is False the f0 stencil is folded into the mask bias
    (filters proportional case) and the A0 pass is skipped entirely.
    """
    import concourse.bacc as bacc
    import concourse.bass as bass
    import concourse.mybir as mybir
    import concourse.tile as tile

    f32 = mybir.dt.float32
    f32r = mybir.dt.float32r
    nc = bacc.Bacc("TRN2", target_bir_lowering=False, debug=False)

    wr_ext = nc.declare_dram_parameter("wr", [N, N], f32r, isOutput=False)
    u_ext = nc.declare_dram_parameter("u", [N, SLAB + 2, U_COLS], f32r, isOutput=False)
    m_ext = nc.declare_dram_parameter("m", [N, SLAB + 1, M_COLS], f32r, isOutput=False)
    wp_ext = nc.declare_dram_parameter("wp", [N, 32 * N], f32r, isOutput=False)
    if use_a0:
        wa_ext = nc.declare_dram_parameter("wa", [N, 9 * N], f32r, isOutput=False)
    v_ext = nc.declare_dram_parameter("v", [N, SLAB, N], f32, isOutput=True)

    with tile.TileContext(nc) as tc:
        with (
            tc.tile_pool(name="const", bufs=1) as const,
            tc.tile_pool(name="wpool", bufs=1) as wpool,
            tc.tile_pool(name="psum", bufs=3, space=bass.MemorySpace.PSUM) as psum,
        ):
            wr_sb = const.tile([N, N], f32r, tag="wr")
            u_sb = const.tile([N, SLAB + 2, U_COLS], f32r, tag="u")
            m_sb = const.tile([N, SLAB + 1, M_COLS], f32r, tag="m")
            wp_sb = const.tile([N, 32 * N], f32r, tag="wp")
            ud_sb = const.tile([N, SLAB + 2, U_COLS], f32r, tag="ud")
            v_sb = const.tile([N, SLAB, N], f32, tag="v")

            nc.sync.dma_start(wr_sb[:], wr_ext[:])
            nc.sync.dma_start(u_sb[:, 0:6, :], u_ext[:, 0:6, :])
            nc.sync.dma_start(m_sb[:, 0:5, :], m_ext[:, 0:5, :])
            nc.scalar.dma_start(wp_sb[:, :16 * N], wp_ext[:, :16 * N])
            nc.sync.dma_start(u_sb[:, 6:SLAB + 2, :], u_ext[:, 6:SLAB + 2, :])
            nc.sync.dma_start(m_sb[:, 5:SLAB + 1, :], m_ext[:, 5:SLAB + 1, :])
            nc.scalar.dma_start(wp_sb[:, 16 * N:], wp_ext[:, 16 * N:])
            if use_a0:
                wa_sb = const.tile([N, 9 * N], f32r, tag="wa")
                nc.scalar.dma_start(wa_sb[:], wa_ext[:])

            W = [wpool.tile([N, SLAB + 1, M_COLS], f32r, tag=f"W{b}",
                            name=f"W{b}") for b in range(8)]
            uflat = u_sb.rearrange("p x z -> p (x z)")
            udflat = ud_sb.rearrange("p x z -> p (x z)")

            for rep in range(reps):
                # ---- ud[y] = u[y+1]: circulant shift matmuls ----
                c0 = 0
                while c0 < U_ELEMS:
                    cw = min(512, U_ELEMS - c0)
                    sps = psum.tile([N, cw], f32, tag="spsum",
                                    name=f"sps{rep}_{c0}")
                    nc.tensor.matmul(sps[:], wr_sb[:], uflat[:, c0:c0 + cw],
                                     start=True, stop=True)
                    nc.scalar.copy(udflat[:, c0:c0 + cw], sps[:])
                    c0 += cw

                # ---- W_b = m * U[.+db] (chunked; DVE / GPSIMD split) ----
                for ci, (s0, cnt) in enumerate(W_CHUNKS):
                    for b in B_ORDER:
                        dbx, dby, dbz = (int(v) for v in CORNERS[b])
                        usel = ud_sb if dby else u_sb
                        eng = nc.vector if b in (0, 1, 4, 2, 6) else nc.gpsimd
                        eng.tensor_mul(
                            W[b][:, s0:s0 + cnt, :],
                            m_sb[:, s0:s0 + cnt, :].bitcast(f32),
                            usel[:, s0 + dbx:s0 + dbx + cnt,
                                 dbz:dbz + M_COLS].bitcast(f32))

                # ---- V accumulation: [A0 stencil +] Ptilde scatter convs ----
                for oc in range(4):
                    vps = psum.tile([N, 4, N], f32, tag="vpsum",
                                    name=f"vps{rep}_{oc}")
                    first = True
                    if use_a0:
                        for gi, (dx, dz) in enumerate(A0_GROUPS):
                            rhs = u_sb[:, 4 * oc + 1 + dx:4 * oc + 5 + dx,
                                       dz + 1:dz + 1 + N]
                            nc.tensor.matmul(
                                vps[:], wa_sb[:, gi * N:(gi + 1) * N],
                                rhs, start=first, stop=False)
                            first = False
                    for bi, b in enumerate(B_ORDER):
                        for gidx, (dax, daz) in enumerate(PG):
                            i = bi * 4 + gidx
                            rhs = W[b][:, 4 * oc - dax + 1:4 * oc - dax + 5,
                                       1 - daz:1 - daz + N]
                            nc.tensor.matmul(
                                vps[:], wp_sb[:, i * N:(i + 1) * N], rhs,
                                start=first,
                                stop=(bi == 7 and gidx == len(PG) - 1))
                            first = False
                    nc.scalar.copy(v_sb[:, 4 * oc:4 * oc + 4, :], vps[:])
                    if rep == reps - 1:
                        nc.sync.dma_start(v_ext[:, 4 * oc:4 * oc + 4, :],
                                          v_sb[:, 4 * oc:4 * oc + 4, :])

    nc.compile()
    return nc


_PROGRAM_CACHE = {}


def _get_program(use_a0):
    key = ("nc", use_a0)
    if key not in _PROGRAM_CACHE:
        _PROGRAM_CACHE[key] = build_program(use_a0)
    return _PROGRAM_CACHE[key]


_R_MAT = None


def kernel(U, H8types, filters, _trace=False):
    from concourse.bass_utils import run_bass_kernel_spmd

    U = np.asarray(U)
    H8types = np.asarray(H8types)
    filters = np.asarray(filters)

    global _R_MAT
    if _R_MAT is None:
        _R_MAT = np.ascontiguousarray(_roll_mat(1).astype(np.float32))

    rho = check_proportional(filters)
    use_a0 = rho is None
    slabs = build_slabs(U, H8types, mask_bias=0.0 if use_a0 else rho)
    wp = np.ascontiguousarray(build_weights_p(filters).reshape(N, -1))

    nc = _get_program(use_a0)
    core_ids = list(range(NCORES))
    in_maps = []
    for c in core_ids:
        u, m = slabs[c]
        im = {"wr": _R_MAT, "u": u, "m": m, "wp": wp}
        if use_a0:
            im["wa"] = np.ascontiguousarray(
                build_weights_a0(filters).reshape(N, -1))
        in_maps.append(im)

    res = run_bass_kernel_spmd(nc, in_maps, core_ids, trace=_trace)
    out = np.empty((N, N, N), dtype=np.float32)
    for c in core_ids:
        v = np.asarray(res.results[c]["v"])  # [128(y), 16(x), 128(z)]
        out[c * SLAB:(c + 1) * SLAB] = v.transpose(1, 0, 2)
    if _trace:
        return out, res
    return out
